# revision 5
# baseline (speedup 1.0000x reference)
"""Trainium2 Bass kernel for nn_Classical_autoencoder (patch MLP autoencoder + cosine fold).

Contract: kernel(**inputs) takes FULL inputs (img (32,1,512,512), W1 (16,4), b1 (4,),
W2 (4,4), b2 (4,), W3 (4,16), b3 (16,)) and returns the FULL (32,512,512) output.
Internally: pure data-parallel over 8 NeuronCores, 4 images per core.

Math (per image):
  patches x = im2col(img, 4x4, stride 2)           # (255*255, 16)
  y = relu(relu(relu(x@W1+b1)@W2+b2)@W3+b3)        # (P, 16)
  S[i,j] = x.y / (max(|x|,eps)*max(|y|,eps))       # (255,255)
  out[r,c] = mean of S[i,j] for i in {r//2-1, r//2} & [0,255), j likewise
  (the overlapping fold with k=4,s=2 reduces exactly to this 2-tap box filter
   on S, upsampled 2x with 2x2-constant blocks)

Layout on chip (per image):
  row tile RT [128=(32k+g), 8=(li), 2=(t), 256=(jj)] : partition (k,g) holds
      img row 16g+k+2li, de-interleaved by column parity (col = 2jj+t);
  patch (i=8g+li, j) kernel col l lives at t=l%2, jj=j+l//2.
  MLP runs with patches as matmul free dim, 32 groups block-diag weights;
  per-patch contractions (x.y, |x|^2, |y|^2) are ones-block-diag matmuls into
  one PSUM tile [96=(3 x 32g), ...]; cosine via direct-emitted Rsqrt act.
  Passes of 1020 patch-columns (li quartets), each matmul split in two
  510-wide halves (PSUM bank limit).
"""

import sys

for _p in ("/opt/trn_rl_repo", "/root/.axon_site/_ro/trn_rl_repo"):
    if _p not in sys.path:
        sys.path.append(_p)

from contextlib import ExitStack

import numpy as np

import concourse.bass as bass
import concourse.tile as tile
from concourse import bacc, mybir

F32 = mybir.dt.float32
BF16 = mybir.dt.bfloat16

IMG = 512
KS = 4
STRIDE = 2
OH = 255  # output patches per dim
NSAMP = 4  # images per core
NCORES = 8


def raw_activation(nc, out, in_, func, bias=0.0, scale=1.0):
    """nc.scalar.activation minus the Rsqrt accuracy guard (measured 4e-5 max
    rel err on HW, far inside this problem's 2e-2 gate)."""
    eng = nc.scalar
    if isinstance(bias, float):
        bias = nc.const_aps.scalar_like(bias, in_)
    inputs = [eng.lower_ap(in_)]
    for arg in (bias, scale, 0.0):
        if isinstance(arg, bass.AP):
            inputs.append(eng.lower_ap(arg))
        else:
            inputs.append(mybir.ImmediateValue(dtype=mybir.dt.float32, value=arg))
    return eng.add_instruction(
        mybir.InstActivation(
            name=nc.get_next_instruction_name(),
            func=func,
            ins=inputs,
            outs=[eng.lower_ap(out)],
        )
    )


def build_nc() -> bass.Bass:
    nc = bacc.Bacc()

    img4b = nc.declare_dram_parameter("img4b", [NSAMP, 128, 8 * IMG], BF16, isOutput=False)[:]
    l1w = nc.declare_dram_parameter("l1w", [128, 4, 128], BF16, isOutput=False)[:]
    l2w = nc.declare_dram_parameter("l2w", [128, 128], BF16, isOutput=False)[:]
    l3w = nc.declare_dram_parameter("l3w", [128, 4, 128], BF16, isOutput=False)[:]
    b3v = nc.declare_dram_parameter("b3v", [128, 4], F32, isOutput=False)[:]
    cw = nc.declare_dram_parameter("cw", [128, 32], BF16, isOutput=False)[:]
    b1v = nc.declare_dram_parameter("b1v", [128, 1], F32, isOutput=False)[:]
    b2v = nc.declare_dram_parameter("b2v", [128, 1], F32, isOutput=False)[:]
    out4 = nc.declare_dram_parameter("out4", [NSAMP, IMG, IMG], F32, isOutput=True)[:]

    RELU = mybir.ActivationFunctionType.Relu
    COPY = mybir.ActivationFunctionType.Copy
    RSQRT = mybir.ActivationFunctionType.Rsqrt
    MULT = mybir.AluOpType.mult
    ADD = mybir.AluOpType.add

    with ExitStack() as ctx:
        tc = ctx.enter_context(tile.TileContext(nc))
        consts = ctx.enter_context(tc.tile_pool(name="consts", bufs=1))
        rows = ctx.enter_context(tc.tile_pool(name="rows", bufs=2))
        mlp = ctx.enter_context(tc.tile_pool(name="mlp", bufs=2))
        simp = ctx.enter_context(tc.tile_pool(name="simp", bufs=2))
        foldp = ctx.enter_context(tc.tile_pool(name="foldp", bufs=2))
        pz12 = ctx.enter_context(tc.tile_pool(name="pz12", bufs=2, space="PSUM"))
        pz3 = ctx.enter_context(tc.tile_pool(name="pz3", bufs=1, space="PSUM"))
        pct = ctx.enter_context(tc.tile_pool(name="pct", bufs=1, space="PSUM"))
        dram = ctx.enter_context(tc.tile_pool(name="dram", bufs=2, space="DRAM"))

        # ---- constants ----
        l1w_t = consts.tile([128, 4, 128], BF16)
        nc.sync.dma_start(out=l1w_t, in_=l1w[:, :, :])
        l2w_t = consts.tile([128, 128], BF16)
        nc.sync.dma_start(out=l2w_t, in_=l2w[:, :])
        l3w_t = consts.tile([128, 4, 128], BF16)
        nc.sync.dma_start(out=l3w_t, in_=l3w[:, :, :])
        b3_t = consts.tile([128, 4], F32)
        nc.sync.dma_start(out=b3_t, in_=b3v[:, :])
        cw_t = consts.tile([128, 32], BF16)
        nc.sync.dma_start(out=cw_t, in_=cw[:, :])
        b1_t = consts.tile([128, 1], F32)
        nc.sync.dma_start(out=b1_t, in_=b1v[:, :])
        b2_t = consts.tile([128, 1], F32)
        nc.sync.dma_start(out=b2_t, in_=b2v[:, :])
        eps_t = consts.tile([32, 1], F32)
        nc.vector.memset(eps_t, 1e-20)

        rtbs = [None] * NSAMP

        def load_sample(s):
            rtb = rows.tile([128, 8, 2, 256], BF16, tag="rtb")
            nc.sync.dma_start(
                out=rtb.rearrange("p a t j -> p (a t j)"), in_=img4b[s, :, :]
            )
            rtbs[s] = rtb

        load_sample(0)

        # deferred DVE fold work (upsample ops) injected into the next pass
        pending_dve = []

        def mm_pass(s, ci2):
            """One 1020-column pass: li in [4*ci2, 4*ci2+4), both halves."""
            rtb = rtbs[s]
            rtbr = rtb.rearrange("p (c h lh) t j -> p c h lh t j", c=2, h=2)

            def xvh(h, l):  # matmul rhs half [128, 2, 255]
                return rtbr[:, ci2, h, :, l % 2, (l // 2) : (l // 2) + 255]

            def xv4(l):  # DVE operand [128, 2, 2, 255]
                return rtbr[:, ci2, :, :, l % 2, (l // 2) : (l // 2) + 255]

            # squared inputs for |x|^2 (bf16, this pass's half of the image)
            i2 = mlp.tile([128, 4, 2, 256], BF16, tag="i2")
            nc.vector.tensor_tensor(i2, rtbr[:, ci2], rtbr[:, ci2], MULT)
            i2r = i2.rearrange("p (h lh) t j -> p h lh t j", h=2)

            def i2vh(h, l):
                return i2r[:, h, :, l % 2, (l // 2) : (l // 2) + 255]

            # ---- layer 1 ----
            z12 = pz12.tile([128, 2, 512], F32, tag="z12")
            for l in range(4):
                for h in range(2):
                    nc.tensor.matmul(
                        z12[:, h, 0:510], l1w_t[:, l, :], xvh(h, l),
                        start=(l == 0), stop=(l == 3),
                    )
            h1 = mlp.tile([128, 2, 510], BF16, tag="h1")
            nc.scalar.activation(h1, z12[:, :, 0:510], RELU, bias=b1_t[:, :])
            # ---- layer 2 (reuses z12's banks after h1 consumed them) ----
            for h in range(2):
                nc.tensor.matmul(
                    z12[:, h, 0:510], l2w_t[:, :], h1[:, h, :], start=True, stop=True
                )
            h2 = mlp.tile([128, 2, 510], BF16, tag="h2")
            nc.scalar.activation(h2, z12[:, :, 0:510], RELU, bias=b2_t[:, :])

            # ---- contraction PSUM tile: [0:32]=x.y, [32:64]=|x|^2, [64:96]=|y|^2
            ct = pct.tile([96, 2, 512], F32, tag="ct")
            # |x|^2 block early: only needs i2, keeps PE busy while h2/yv run
            for l in range(4):
                for h in range(2):
                    nc.tensor.matmul(
                        ct[32:64, h, 0:510], cw_t, i2vh(h, l),
                        start=(l == 0), stop=(l == 3),
                    )

            ysqs = []
            first_dve = True
            for l in range(4):
                z3 = pz3.tile([128, 2, 512], F32, tag="z3")
                for h in range(2):
                    nc.tensor.matmul(
                        z3[:, h, 0:510], l3w_t[:, l, :], h2[:, h, :],
                        start=True, stop=True,
                    )
                yv = mlp.tile([128, 2, 2, 255], BF16, tag="yv")
                nc.scalar.activation(
                    yv, z3[:, :, 0:510].rearrange("p h (lh j) -> p h lh j", lh=2),
                    RELU, bias=b3_t[:, l : l + 1],
                )
                prod = mlp.tile([128, 2, 2, 255], BF16, tag="prod")
                nc.vector.tensor_tensor(prod, yv, xv4(l), MULT)
                if first_dve and pending_dve:
                    # inject deferred fold ops while their bounce DMAs are done
                    for fn in pending_dve:
                        fn()
                    pending_dve.clear()
                    first_dve = False
                ysq = mlp.tile([128, 2, 2, 255], BF16, tag=f"ysq{l}")
                eng = nc.gpsimd if l < 2 else nc.vector
                eng.tensor_tensor(ysq, yv, yv, MULT)
                ysqs.append(ysq)
                prodr = prod.rearrange("p h lh j -> p h (lh j)")
                for h in range(2):
                    nc.tensor.matmul(
                        ct[0:32, h, 0:510], cw_t, prodr[:, h, :],
                        start=(l == 0), stop=(l == 3),
                    )
            for l in range(4):
                yr = ysqs[l].rearrange("p h lh j -> p h (lh j)")
                for h in range(2):
                    nc.tensor.matmul(
                        ct[64:96, h, 0:510], cw_t, yr[:, h, :],
                        start=(l == 0), stop=(l == 3),
                    )

            # ---- cosine similarity, scaled by 1/4 for the fold ----
            # 1/(4*sqrt(ctx*cty)) = Rsqrt(16*ctx) * Rsqrt(cty); two scalar
            # acts read PSUM directly (DVE can't read two PSUM operands)
            rs1 = simp.tile([32, 2, 510], F32, tag="rs1")
            raw_activation(nc, rs1, ct[32:64, :, 0:510], RSQRT, bias=eps_t[:, :], scale=16.0)
            rs2 = simp.tile([32, 2, 510], F32, tag="rs2")
            raw_activation(nc, rs2, ct[64:96, :, 0:510], RSQRT, bias=eps_t[:, :])
            st = simp.tile([32, 2, 510], F32, tag="st")
            nc.vector.tensor_tensor(st, ct[0:32, :, 0:510], rs1, MULT)
            return st, rs2

        for s in range(NSAMP):
            if s + 1 < NSAMP:
                load_sample(s + 1)
            sim32 = simp.tile([32, 2, 2, 510], F32, tag="sim32")
            for ci2 in range(2):
                st, rs2 = mm_pass(s, ci2)
                nc.vector.tensor_tensor(sim32[:, ci2], st, rs2, MULT)

            # ---- reorganize S (g-blocked rows) -> row-pair layout via DRAM ----
            sdram = dram.tile([32 * 8 * OH], F32, tag="sd")
            nc.sync.dma_start(
                out=sdram.rearrange("(g a b j) -> g a b j", g=32, a=2, b=2), in_=sim32
            )
            simt = foldp.tile([128, 2, 256], F32, tag="simt")
            nc.sync.dma_start(
                out=simt[0:128, :, 0:255],
                in_=bass.AP(
                    tensor=sdram.tensor,
                    offset=sdram.offset,
                    ap=[[2 * OH, 128], [OH, 2], [1, OH]],
                ),
            )

            # ---- fold: R[i,v] = S[i,v-1]+S[i,v] (cols), with edge doubling ----
            rf = foldp.tile([128, 2, 256], F32, tag="rf")
            nc.gpsimd.tensor_tensor(
                rf[:, :, 1:255], simt[:, :, 0:254], simt[:, :, 1:255], ADD
            )
            nc.scalar.activation(rf[:, :, 0:1], simt[:, :, 0:1], COPY, scale=2.0)
            nc.scalar.activation(rf[:, :, 255:256], simt[:, :, 254:255], COPY, scale=2.0)
            # S row 255 doesn't exist -> duplicate row 254 so T[255]=2*R[254]
            nc.sync.dma_start(out=rf[127:128, 1, :], in_=rf[127:128, 0, :])
            # partition-shifted copy of odd rows: rfs[q] = R[2q-1] (rfs[0]=R[0])
            rfs = foldp.tile([128, 256], F32, tag="rfs")
            nc.sync.dma_start(out=rfs[1:128, :], in_=rf[0:127, 1, :])
            nc.sync.dma_start(out=rfs[0:1, :], in_=rf[0:1, 0, :])

            # ---- fold rows + upsample 2x2 fused via stride-0 broadcast adds ----
            # up[p, lu, ru, v, cv] -> out row 4p+2lu+ru, col 2v+cv
            up = foldp.tile([128, 2, 2, 256, 2], F32, tag="up")
            s_ = s

            def emit_up(up=up, rf=rf, rfs=rfs, s=s_):
                def bc(a):
                    return a.unsqueeze(1).unsqueeze(3).to_broadcast([128, 2, 256, 2])

                nc.vector.tensor_tensor(up[:, 1], bc(rf[:, 0, :]), bc(rf[:, 1, :]), ADD)
                nc.vector.tensor_tensor(up[:, 0], bc(rfs[:, :]), bc(rf[:, 0, :]), ADD)
                nc.sync.dma_start(
                    out=bass.AP(
                        tensor=out4.tensor,
                        offset=out4.offset + s * IMG * IMG,
                        ap=[[4 * IMG, 128], [2 * IMG, 2], [IMG, 2], [2, 256], [1, 2]],
                    ),
                    in_=up,
                )

            if s + 1 < NSAMP:
                pending_dve.append(emit_up)
            else:
                emit_up()

    nc.finalize()
    return nc


def make_weight_inputs(W1, b1, W2, b2, W3, b3):
    """Host-side block-diagonal weight construction (all fp32)."""
    W1 = np.asarray(W1, np.float32)
    W2 = np.asarray(W2, np.float32)
    W3 = np.asarray(W3, np.float32)
    b1 = np.asarray(b1, np.float32)
    b2 = np.asarray(b2, np.float32)
    b3 = np.asarray(b3, np.float32)
    # partition orders: image/z3 rows p = 32k+g ; h1/h2 rows q = 32c+g
    l1w = np.zeros((128, 4, 128), np.float32)
    l2w = np.zeros((128, 128), np.float32)
    l3w = np.zeros((128, 4, 128), np.float32)
    b3v = np.zeros((128, 4), np.float32)
    cwm = np.zeros((128, 32), np.float32)
    for g in range(32):
        for l in range(4):
            for k in range(4):
                for c in range(4):
                    l1w[32 * k + g, l, 32 * c + g] = W1[4 * k + l, c]
                    l3w[32 * c + g, l, 32 * k + g] = W3[c, 4 * k + l]
                b3v[32 * k + g, l] = b3[4 * k + l]
                cwm[32 * k + g, g] = 1.0
        for c in range(4):
            for c2 in range(4):
                l2w[32 * c + g, 32 * c2 + g] = W2[c, c2]
    b1v = np.repeat(b1, 32).reshape(128, 1).astype(np.float32)
    b2v = np.repeat(b2, 32).reshape(128, 1).astype(np.float32)
    import ml_dtypes

    bf = ml_dtypes.bfloat16
    return {
        "l1w": l1w.astype(bf), "l2w": l2w.astype(bf), "l3w": l3w.astype(bf),
        "b3v": b3v, "cw": cwm.astype(bf), "b1v": b1v, "b2v": b2v,
    }


_NC = None


def get_nc():
    global _NC
    if _NC is None:
        _NC = build_nc()
    return _NC


def _bf16():
    import ml_dtypes

    return ml_dtypes.bfloat16


def gather_rows(img_n):
    """(n,512,512) f32 -> (n,128,4096) bf16: row-gathered and col-parity
    de-interleaved on-chip layout [p, li, t, jj] with col = 2jj+t."""
    n = img_n.shape[0]
    pad = np.zeros((n, IMG + 4, IMG), np.float32)
    pad[:, :IMG, :] = img_n
    p = np.arange(128)
    li = np.arange(8)
    rows_idx = 16 * (p[:, None] % 32) + (p[:, None] // 32) + 2 * li[None, :]
    out = pad[:, rows_idx, :]  # (n,128,8,512)
    out = out.reshape(n, 128, 8, 256, 2).transpose(0, 1, 2, 4, 3)  # (n,128,8,2,256)
    return np.ascontiguousarray(out.reshape(n, 128, 8 * IMG)).astype(_bf16())


def kernel(img, W1, b1, W2, b2, W3, b3):
    from concourse.bass_utils import run_bass_kernel_spmd

    img = np.asarray(img, np.float32).reshape(32, IMG, IMG)
    wts = make_weight_inputs(W1, b1, W2, b2, W3, b3)
    nc = get_nc()
    core_ids = list(range(NCORES))
    in_maps = []
    for c in range(NCORES):
        m = {"img4b": gather_rows(img[c * NSAMP : (c + 1) * NSAMP])}
        m.update(wts)
        in_maps.append(m)
    res = run_bass_kernel_spmd(nc, in_maps, core_ids)
    out = np.concatenate([res.results[i]["out4"] for i in range(NCORES)], axis=0)
    return out.astype(np.float32)


# revision 13
# speedup vs baseline: 1.1702x; 1.1702x over previous
"""Trainium2 Bass kernel for nn_Classical_autoencoder (patch MLP autoencoder + cosine fold).

Contract: kernel(**inputs) takes FULL inputs (img (32,1,512,512), W1 (16,4), b1 (4,),
W2 (4,4), b2 (4,), W3 (4,16), b3 (16,)) and returns the FULL (32,512,512) output.
Internally: pure data-parallel over 8 NeuronCores, 4 images per core.

Math (per image):
  patches x = im2col(img, 4x4, stride 2)           # (255*255, 16)
  y = relu(relu(relu(x@W1+b1)@W2+b2)@W3+b3)        # (P, 16)
  S[i,j] = x.y / (max(|x|,eps)*max(|y|,eps))       # (255,255)
  out[r,c] = mean of S[i,j] for i in {r//2-1, r//2} & [0,255), j likewise
  (the overlapping fold with k=4,s=2 reduces exactly to this 2-tap box filter
   on S, upsampled 2x with 2x2-constant blocks)

Layout on chip (per image):
  row tile RT [128=(32k+g), 8=(li), 2=(t), 256=(jj)] : partition (k,g) holds
      img row 16g+k+2li, de-interleaved by column parity (col = 2jj+t);
  patch (i=8g+li, j) kernel col l lives at t=l%2, jj=j+l//2.
  MLP runs with patches as matmul free dim, 32 groups block-diag weights;
  per-patch contractions (x.y, |x|^2, |y|^2) are ones-block-diag matmuls into
  one PSUM tile [96=(3 x 32g), ...]; cosine via direct-emitted Rsqrt act.
  Two 1020-column passes per image (li quartets); each matmul is split into
  two 510-wide halves (PSUM bank limit); each pass's similarity tail is
  software-pipelined into the next pass so the PE never drains.
"""

import sys

for _p in ("/opt/trn_rl_repo", "/root/.axon_site/_ro/trn_rl_repo"):
    if _p not in sys.path:
        sys.path.append(_p)

from contextlib import ExitStack

import numpy as np

import concourse.bass as bass
import concourse.tile as tile
from concourse import bacc, mybir

F32 = mybir.dt.float32
BF16 = mybir.dt.bfloat16

IMG = 512
KS = 4
STRIDE = 2
OH = 255  # output patches per dim
NSAMP = 4  # images per core
NCORES = 8


def raw_activation(nc, out, in_, func, bias=0.0, scale=1.0):
    """nc.scalar.activation minus the Rsqrt accuracy guard (measured 4e-5 max
    rel err on HW, far inside this problem's 2e-2 gate)."""
    eng = nc.scalar
    if isinstance(bias, float):
        bias = nc.const_aps.scalar_like(bias, in_)
    inputs = [eng.lower_ap(in_)]
    for arg in (bias, scale, 0.0):
        if isinstance(arg, bass.AP):
            inputs.append(eng.lower_ap(arg))
        else:
            inputs.append(mybir.ImmediateValue(dtype=mybir.dt.float32, value=arg))
    return eng.add_instruction(
        mybir.InstActivation(
            name=nc.get_next_instruction_name(),
            func=func,
            ins=inputs,
            outs=[eng.lower_ap(out)],
        )
    )


def build_nc() -> bass.Bass:
    nc = bacc.Bacc()

    img4b = nc.declare_dram_parameter("img4b", [NSAMP, 128, 8 * IMG], BF16, isOutput=False)[:]
    l1w = nc.declare_dram_parameter("l1w", [128, 4, 128], BF16, isOutput=False)[:]
    l2w = nc.declare_dram_parameter("l2w", [128, 128], BF16, isOutput=False)[:]
    l3w = nc.declare_dram_parameter("l3w", [128, 4, 128], BF16, isOutput=False)[:]
    b3v = nc.declare_dram_parameter("b3v", [128, 4], F32, isOutput=False)[:]
    cw = nc.declare_dram_parameter("cw", [128, 32], BF16, isOutput=False)[:]
    b1v = nc.declare_dram_parameter("b1v", [128, 1], F32, isOutput=False)[:]
    b2v = nc.declare_dram_parameter("b2v", [128, 1], F32, isOutput=False)[:]
    out4 = nc.declare_dram_parameter("out4", [NSAMP, IMG, IMG], F32, isOutput=True)[:]

    RELU = mybir.ActivationFunctionType.Relu
    COPY = mybir.ActivationFunctionType.Copy
    RSQRT = mybir.ActivationFunctionType.Rsqrt
    MULT = mybir.AluOpType.mult
    ADD = mybir.AluOpType.add

    with ExitStack() as ctx:
        tc = ctx.enter_context(tile.TileContext(nc))
        consts = ctx.enter_context(tc.tile_pool(name="consts", bufs=1))
        rows = ctx.enter_context(tc.tile_pool(name="rows", bufs=2))
        mlp = ctx.enter_context(tc.tile_pool(name="mlp", bufs=4))
        simp = ctx.enter_context(tc.tile_pool(name="simp", bufs=2))
        foldp = ctx.enter_context(tc.tile_pool(name="foldp", bufs=2))
        pz = ctx.enter_context(tc.tile_pool(name="pz", bufs=2, space="PSUM"))
        pct = ctx.enter_context(tc.tile_pool(name="pct", bufs=2, space="PSUM"))
        dram = ctx.enter_context(tc.tile_pool(name="dram", bufs=2, space="DRAM"))

        # ---- constants ----
        l1w_t = consts.tile([128, 4, 128], BF16)
        nc.sync.dma_start(out=l1w_t, in_=l1w[:, :, :])
        l2w_t = consts.tile([128, 128], BF16)
        nc.sync.dma_start(out=l2w_t, in_=l2w[:, :])
        l3w_t = consts.tile([128, 4, 128], BF16)
        nc.sync.dma_start(out=l3w_t, in_=l3w[:, :, :])
        b3_t = consts.tile([128, 4], F32)
        nc.sync.dma_start(out=b3_t, in_=b3v[:, :])
        cw_t = consts.tile([128, 32], BF16)
        nc.sync.dma_start(out=cw_t, in_=cw[:, :])
        b1_t = consts.tile([128, 1], F32)
        nc.sync.dma_start(out=b1_t, in_=b1v[:, :])
        b2_t = consts.tile([128, 1], F32)
        nc.sync.dma_start(out=b2_t, in_=b2v[:, :])
        eps_t = consts.tile([32, 1], F32)
        nc.vector.memset(eps_t, 1e-20)

        rtbs = [None] * NSAMP
        sim32s = [None] * NSAMP

        def load_sample(s):
            rtb = rows.tile([128, 8, 2, 256], BF16, tag="rtb", bufs=2)
            nc.sync.dma_start(
                out=rtb.rearrange("p a t j -> p (a t j)"), in_=img4b[s, :, :]
            )
            rtbs[s] = rtb

        load_sample(0)
        load_sample(1)

        # ---------------- deferred emission state ----------------
        pending_tail = []  # closures: prev pass's cty mms + rsqrt/sim tail
        pending_fold = []  # closures: finished sample's bounce + fold setup
        pending_up = []    # closures: fold's upsample DVE ops + output DMA

        def emit_pass(s, ci2):
            rtb = rtbs[s]
            rtbr = rtb.rearrange("p (c h lh) t j -> p c h lh t j", c=2, h=2)

            def xvh(h, l):  # matmul rhs half [128, 2, 255]
                return rtbr[:, ci2, h, :, l % 2, (l // 2) : (l // 2) + 255]

            def xv4(l):  # DVE operand [128, 2, 2, 255]
                return rtbr[:, ci2, :, :, l % 2, (l // 2) : (l // 2) + 255]

            # ---- layer 1 (PE: fully independent of prev pass tail) ----
            z12 = pz.tile([128, 2, 512], F32, tag="z")
            for l in range(4):
                for h in range(2):
                    nc.tensor.matmul(
                        z12[:, h, 0:510], l1w_t[:, l, :], xvh(h, l),
                        start=(l == 0), stop=(l == 3),
                    )
            h1 = mlp.tile([128, 2, 512], BF16, tag="h1", bufs=2)
            nc.scalar.activation(h1, z12, RELU, bias=b1_t[:, :])

            # ---- squared inputs for |x|^2 (flat contiguous bf16) ----
            i2 = mlp.tile([128, 4, 2, 256], BF16, tag="i2", bufs=2)
            rtbf = rtb.rearrange("p a t j -> p (a t j)")
            nc.vector.tensor_tensor(
                i2.rearrange("p a t j -> p (a t j)"),
                rtbf[:, 2048 * ci2 : 2048 * (ci2 + 1)],
                rtbf[:, 2048 * ci2 : 2048 * (ci2 + 1)],
                MULT,
            )

            # ---- previous pass's tail: cty mms (PE), rsqrt+sim (Act/DVE) ----
            for fn in pending_tail:
                fn()
            pending_tail.clear()
            # ---- finished sample's fold front half (DMA/Pool/Act) ----
            for fn in pending_fold:
                fn()
            pending_fold.clear()

            # ---- layer 2 (reuses z12 banks after h1 consumed them) ----
            for h in range(2):
                nc.tensor.matmul(
                    z12[:, h, 0:510], l2w_t[:, :], h1[:, h, 0:510],
                    start=True, stop=True,
                )
            h2 = mlp.tile([128, 2, 512], BF16, tag="h2", bufs=2)
            nc.scalar.activation(h2, z12, RELU, bias=b2_t[:, :])

            # ---- contractions PSUM: [0:32]=x.y, [32:64]=|x|^2, [64:96]=|y|^2
            ct = pct.tile([96, 2, 512], F32, tag="ct")

            def z3mm(lw):
                z3 = pz.tile([128, 2, 512], F32, tag="z")
                for h in range(2):
                    nc.tensor.matmul(
                        z3[:, h, 0:510], l3w_t[:, lw, :], h2[:, h, 0:510],
                        start=True, stop=True,
                    )
                return z3

            z3 = z3mm(0)
            # |x|^2 block: only needs i2 — dense PE filler while yv-l0 runs
            i2r = i2.rearrange("p (h lh) t j -> p h lh t j", h=2)
            for ll in range(4):
                for h in range(2):
                    nc.tensor.matmul(
                        ct[32:64, h, 0:510], cw_t,
                        i2r[:, h, :, ll % 2, (ll // 2) : (ll // 2) + 255],
                        start=(ll == 0), stop=(ll == 3),
                    )

            ysqs = []
            for l in range(4):
                # full-width contiguous act read; yv keeps z3's packed layout
                # (valid halves are 510 = (lh 2, j 255) packed + 2 pad cols)
                yv = mlp.tile([128, 2, 512], BF16, tag="yv", bufs=4)
                nc.scalar.activation(yv, z3, RELU, bias=b3_t[:, l : l + 1])
                prod = mlp.tile([128, 2, 2, 255], BF16, tag="prod", bufs=2)
                nc.vector.tensor_tensor(
                    prod,
                    yv[:, :, 0:510].rearrange("p h (lh j) -> p h lh j", lh=2),
                    xv4(l), MULT,
                )
                ysq = mlp.tile([128, 2, 512], BF16, tag=f"ysq{l}", bufs=2)
                eng = nc.gpsimd if l < 2 else nc.vector
                eng.tensor_tensor(
                    ysq.rearrange("p h x -> p (h x)"),
                    yv.rearrange("p h x -> p (h x)"),
                    yv.rearrange("p h x -> p (h x)"),
                    MULT,
                )
                ysqs.append(ysq)
                if l < 3:
                    z3 = z3mm(l + 1)  # next l's z3 (deps via pool rotation)
                prodr = prod.rearrange("p h lh j -> p h (lh j)")
                for h in range(2):
                    nc.tensor.matmul(
                        ct[0:32, h, 0:510], cw_t, prodr[:, h, :],
                        start=(l == 0), stop=(l == 3),
                    )

            sim32 = sim32s[s]

            def tail(ct=ct, ysqs=ysqs, sim32=sim32, ci2=ci2):
                for l in range(4):
                    for h in range(2):
                        nc.tensor.matmul(
                            ct[64:96, h, 0:510], cw_t, ysqs[l][:, h, 0:510],
                            start=(l == 0), stop=(l == 3),
                        )
                # 1/(4*sqrt(ctx*cty)) = Rsqrt(16*ctx) * Rsqrt(cty)
                rs1 = simp.tile([32, 2, 512], F32, tag="rs1")
                raw_activation(nc, rs1, ct[32:64], RSQRT, bias=eps_t[:, :], scale=16.0)
                rs2 = simp.tile([32, 2, 512], F32, tag="rs2")
                raw_activation(nc, rs2, ct[64:96], RSQRT, bias=eps_t[:, :])
                st = simp.tile([32, 2, 512], F32, tag="st")
                nc.vector.tensor_tensor(
                    st.rearrange("g h x -> g (h x)"),
                    ct[0:32].rearrange("g h x -> g (h x)"),
                    rs1.rearrange("g h x -> g (h x)"),
                    MULT,
                )
                nc.vector.tensor_tensor(
                    sim32[:, ci2].rearrange("g h x -> g (h x)"),
                    st.rearrange("g h x -> g (h x)"),
                    rs2.rearrange("g h x -> g (h x)"),
                    MULT,
                )

            pending_tail.append(tail)

        def emit_fold(s):
            """Bounce S through DRAM into row-pair layout and fold. Emitted
            one pass after sample s's last sim write; the upsample half is
            deferred one further step via pending_up."""
            sim32 = sim32s[s]
            sdram = dram.tile([32 * 2048], F32, tag="sd")
            nc.sync.dma_start(
                out=sdram.rearrange("(g x) -> g x", g=32),
                in_=sim32.rearrange("g a b x -> g (a b x)"),
            )
            # partition p holds S rows 2p,2p+1 (cols 0..254); row pitch 512
            # inside each 4-row group block, pair rows at +0 / +255.
            simt = foldp.tile([128, 2, 256], F32, tag="simt")
            nc.sync.dma_start(
                out=simt[0:128, :, 0:255],
                in_=bass.AP(
                    tensor=sdram.tensor,
                    offset=sdram.offset,
                    ap=[[512, 128], [255, 2], [1, 255]],
                ),
            )
            # col fold R[i,v] = S[i,v-1]+S[i,v], edges doubled
            rf = foldp.tile([128, 2, 256], F32, tag="rf")
            nc.gpsimd.tensor_tensor(
                rf[:, :, 1:255], simt[:, :, 0:254], simt[:, :, 1:255], ADD
            )
            nc.scalar.activation(rf[:, :, 0:1], simt[:, :, 0:1], COPY, scale=2.0)
            nc.scalar.activation(rf[:, :, 255:256], simt[:, :, 254:255], COPY, scale=2.0)
            # S row 255 doesn't exist -> duplicate row 254 so T[255]=2*R[254]
            nc.sync.dma_start(out=rf[127:128, 1, :], in_=rf[127:128, 0, :])
            # partition-shifted copy of odd rows: rfs[q] = R[2q-1] (rfs[0]=R[0])
            rfs = foldp.tile([128, 256], F32, tag="rfs")
            nc.sync.dma_start(out=rfs[1:128, :], in_=rf[0:127, 1, :])
            nc.sync.dma_start(out=rfs[0:1, :], in_=rf[0:1, 0, :])

            def up_ops(rf=rf, rfs=rfs, s=s):
                # row fold + 2x2 upsample fused via stride-0 broadcast adds
                # up[p, lu, ru, v, cv] -> out row 4p+2lu+ru, col 2v+cv
                up = foldp.tile([128, 2, 2, 256, 2], F32, tag="up")

                def bc(a):
                    return a.unsqueeze(1).unsqueeze(3).to_broadcast([128, 2, 256, 2])

                nc.vector.tensor_tensor(up[:, 1], bc(rf[:, 0, :]), bc(rf[:, 1, :]), ADD)
                nc.vector.tensor_tensor(up[:, 0], bc(rfs[:, :]), bc(rf[:, 0, :]), ADD)
                nc.sync.dma_start(
                    out=bass.AP(
                        tensor=out4.tensor,
                        offset=out4.offset + s * IMG * IMG,
                        ap=[[4 * IMG, 128], [2 * IMG, 2], [IMG, 2], [2, 256], [1, 2]],
                    ),
                    in_=up,
                )

            pending_up.append(up_ops)

        # ---------------- main schedule ----------------
        for s in range(NSAMP):
            sim32s[s] = simp.tile([32, 2, 2, 512], F32, tag="sim32", name="sim32")
            if s >= 1 and s + 1 < NSAMP:
                load_sample(s + 1)
            for ci2 in range(2):
                emit_pass(s, ci2)
                if pending_up:
                    for fn in pending_up:
                        fn()
                    pending_up.clear()
                if s > 0 and ci2 == 0:
                    emit_fold(s - 1)
        for fn in pending_tail:
            fn()
        pending_tail.clear()
        emit_fold(NSAMP - 1)
        for fn in pending_up:
            fn()
        pending_up.clear()

    nc.finalize()
    return nc


def make_weight_inputs(W1, b1, W2, b2, W3, b3):
    """Host-side block-diagonal weight construction (all fp32)."""
    W1 = np.asarray(W1, np.float32)
    W2 = np.asarray(W2, np.float32)
    W3 = np.asarray(W3, np.float32)
    b1 = np.asarray(b1, np.float32)
    b2 = np.asarray(b2, np.float32)
    b3 = np.asarray(b3, np.float32)
    # partition orders: image/z3 rows p = 32k+g ; h1/h2 rows q = 32c+g
    l1w = np.zeros((128, 4, 128), np.float32)
    l2w = np.zeros((128, 128), np.float32)
    l3w = np.zeros((128, 4, 128), np.float32)
    b3v = np.zeros((128, 4), np.float32)
    cwm = np.zeros((128, 32), np.float32)
    for g in range(32):
        for l in range(4):
            for k in range(4):
                for c in range(4):
                    l1w[32 * k + g, l, 32 * c + g] = W1[4 * k + l, c]
                    l3w[32 * c + g, l, 32 * k + g] = W3[c, 4 * k + l]
                b3v[32 * k + g, l] = b3[4 * k + l]
                cwm[32 * k + g, g] = 1.0
        for c in range(4):
            for c2 in range(4):
                l2w[32 * c + g, 32 * c2 + g] = W2[c, c2]
    b1v = np.repeat(b1, 32).reshape(128, 1).astype(np.float32)
    b2v = np.repeat(b2, 32).reshape(128, 1).astype(np.float32)
    import ml_dtypes

    bf = ml_dtypes.bfloat16
    return {
        "l1w": l1w.astype(bf), "l2w": l2w.astype(bf), "l3w": l3w.astype(bf),
        "b3v": b3v, "cw": cwm.astype(bf), "b1v": b1v, "b2v": b2v,
    }


_NC = None


def get_nc():
    global _NC
    if _NC is None:
        _NC = build_nc()
    return _NC


def _bf16():
    import ml_dtypes

    return ml_dtypes.bfloat16


def gather_rows(img_n):
    """(n,512,512) f32 -> (n,128,4096) bf16: row-gathered and col-parity
    de-interleaved on-chip layout [p, li, t, jj] with col = 2jj+t."""
    n = img_n.shape[0]
    pad = np.zeros((n, IMG + 4, IMG), np.float32)
    pad[:, :IMG, :] = img_n
    p = np.arange(128)
    li = np.arange(8)
    rows_idx = 16 * (p[:, None] % 32) + (p[:, None] // 32) + 2 * li[None, :]
    out = pad[:, rows_idx, :]  # (n,128,8,512)
    out = out.reshape(n, 128, 8, 256, 2).transpose(0, 1, 2, 4, 3)  # (n,128,8,2,256)
    return np.ascontiguousarray(out.reshape(n, 128, 8 * IMG)).astype(_bf16())


def kernel(img, W1, b1, W2, b2, W3, b3):
    from concourse.bass_utils import run_bass_kernel_spmd

    img = np.asarray(img, np.float32).reshape(32, IMG, IMG)
    wts = make_weight_inputs(W1, b1, W2, b2, W3, b3)
    nc = get_nc()
    core_ids = list(range(NCORES))
    in_maps = []
    for c in range(NCORES):
        m = {"img4b": gather_rows(img[c * NSAMP : (c + 1) * NSAMP])}
        m.update(wts)
        in_maps.append(m)
    res = run_bass_kernel_spmd(nc, in_maps, core_ids)
    out = np.concatenate([res.results[i]["out4"] for i in range(NCORES)], axis=0)
    return out.astype(np.float32)


# revision 23
# speedup vs baseline: 1.2389x; 1.0587x over previous
"""Trainium2 Bass kernel for nn_Classical_autoencoder (patch MLP autoencoder + cosine fold).

Contract: kernel(**inputs) takes FULL inputs (img (32,1,512,512), W1 (16,4), b1 (4,),
W2 (4,4), b2 (4,), W3 (4,16), b3 (16,)) and returns the FULL (32,512,512) output.
Internally: pure data-parallel over 8 NeuronCores, 4 images per core.

Math (per image):
  patches x = im2col(img, 4x4, stride 2)           # (255*255, 16)
  y = relu(relu(relu(x@W1+b1)@W2+b2)@W3+b3)        # (P, 16)
  S[i,j] = x.y / (max(|x|,eps)*max(|y|,eps))       # (255,255)
  out[r,c] = mean of S[i,j] for i in {r//2-1, r//2} & [0,255), j likewise
  (the overlapping fold with k=4,s=2 reduces exactly to this 2-tap box filter
   on S, upsampled 2x with 2x2-constant blocks)

Layout on chip (per image):
  row tile RT [128=(32k+g), 8=(li), 2=(t), 256=(jj)] : partition (k,g) holds
      img row 16g+k+2li, de-interleaved by column parity (col = 2jj+t);
  patch (i=8g+li, j) kernel col l lives at t=l%2, jj=j+l//2.
  MLP runs with patches as matmul free dim, 32 groups block-diag weights;
  per-patch contractions (x.y, |x|^2, |y|^2) are ones-block-diag matmuls into
  one PSUM tile [96=(3 x 32g), ...]; cosine via direct-emitted Rsqrt act.
  Two 1020-column passes per image (li quartets); each matmul is split into
  two 510-wide halves (PSUM bank limit); each pass's similarity tail is
  software-pipelined into the next pass so the PE never drains.
"""

import sys

for _p in ("/opt/trn_rl_repo", "/root/.axon_site/_ro/trn_rl_repo"):
    if _p not in sys.path:
        sys.path.append(_p)

from contextlib import ExitStack

import numpy as np

import concourse.bass as bass
import concourse.tile as tile
from concourse import bacc, mybir

F32 = mybir.dt.float32
BF16 = mybir.dt.bfloat16

IMG = 512
KS = 4
STRIDE = 2
OH = 255  # output patches per dim
NSAMP = 4  # images per core
NCORES = 8


def raw_activation(nc, out, in_, func, bias=0.0, scale=1.0):
    """nc.scalar.activation minus the Rsqrt accuracy guard (measured 4e-5 max
    rel err on HW, far inside this problem's 2e-2 gate)."""
    eng = nc.scalar
    if isinstance(bias, float):
        bias = nc.const_aps.scalar_like(bias, in_)
    inputs = [eng.lower_ap(in_)]
    for arg in (bias, scale, 0.0):
        if isinstance(arg, bass.AP):
            inputs.append(eng.lower_ap(arg))
        else:
            inputs.append(mybir.ImmediateValue(dtype=mybir.dt.float32, value=arg))
    return eng.add_instruction(
        mybir.InstActivation(
            name=nc.get_next_instruction_name(),
            func=func,
            ins=inputs,
            outs=[eng.lower_ap(out)],
        )
    )


def build_nc() -> bass.Bass:
    nc = bacc.Bacc()

    img4b = nc.declare_dram_parameter("img4b", [NSAMP, 128, 8 * IMG], BF16, isOutput=False)[:]
    l1w = nc.declare_dram_parameter("l1w", [128, 4, 128], BF16, isOutput=False)[:]
    l2w = nc.declare_dram_parameter("l2w", [128, 128], BF16, isOutput=False)[:]
    l3w = nc.declare_dram_parameter("l3w", [128, 4, 128], BF16, isOutput=False)[:]
    b3v = nc.declare_dram_parameter("b3v", [128, 4], F32, isOutput=False)[:]
    cw = nc.declare_dram_parameter("cw", [128, 32], BF16, isOutput=False)[:]
    b1v = nc.declare_dram_parameter("b1v", [128, 1], F32, isOutput=False)[:]
    b2v = nc.declare_dram_parameter("b2v", [128, 1], F32, isOutput=False)[:]
    out4 = nc.declare_dram_parameter("out4", [NSAMP, IMG, IMG], F32, isOutput=True)[:]

    RELU = mybir.ActivationFunctionType.Relu
    COPY = mybir.ActivationFunctionType.Copy
    RSQRT = mybir.ActivationFunctionType.Rsqrt
    MULT = mybir.AluOpType.mult
    ADD = mybir.AluOpType.add

    with ExitStack() as ctx:
        tc = ctx.enter_context(tile.TileContext(nc))
        consts = ctx.enter_context(tc.tile_pool(name="consts", bufs=1))
        rows = ctx.enter_context(tc.tile_pool(name="rows", bufs=2))
        mlp = ctx.enter_context(tc.tile_pool(name="mlp", bufs=4))
        simp = ctx.enter_context(tc.tile_pool(name="simp", bufs=2))
        foldp = ctx.enter_context(tc.tile_pool(name="foldp", bufs=2))
        pz = ctx.enter_context(tc.tile_pool(name="pz", bufs=2, space="PSUM"))
        pct = ctx.enter_context(tc.tile_pool(name="pct", bufs=2, space="PSUM"))
        dram = ctx.enter_context(tc.tile_pool(name="dram", bufs=2, space="DRAM"))

        # ---- constants ----
        l1w_t = consts.tile([128, 4, 128], BF16)
        nc.sync.dma_start(out=l1w_t, in_=l1w[:, :, :])
        l2w_t = consts.tile([128, 128], BF16)
        nc.sync.dma_start(out=l2w_t, in_=l2w[:, :])
        l3w_t = consts.tile([128, 4, 128], BF16)
        nc.sync.dma_start(out=l3w_t, in_=l3w[:, :, :])
        b3_t = consts.tile([128, 4], F32)
        nc.sync.dma_start(out=b3_t, in_=b3v[:, :])
        cw_t = consts.tile([128, 32], BF16)
        nc.sync.dma_start(out=cw_t, in_=cw[:, :])
        b1_t = consts.tile([128, 1], F32)
        nc.sync.dma_start(out=b1_t, in_=b1v[:, :])
        b2_t = consts.tile([128, 1], F32)
        nc.sync.dma_start(out=b2_t, in_=b2v[:, :])
        eps_t = consts.tile([128, 1], F32)
        nc.vector.memset(eps_t, 1e-20)

        rtbs = [None] * NSAMP
        sim32s = [None] * NSAMP

        def load_sample(s):
            rtb = rows.tile([128, 8, 2, 256], BF16, tag="rtb", bufs=2)
            nc.sync.dma_start(
                out=rtb.rearrange("p a t j -> p (a t j)"), in_=img4b[s, :, :]
            )
            rtbs[s] = rtb

        load_sample(0)
        load_sample(1)

        # ---------------- deferred emission state ----------------
        pending_tail = []  # closures: prev pass's cty mms + rsqrt/sim tail
        pending_fold = []  # closures: finished sample's bounce + fold setup
        pending_up = []    # closures: fold's upsample DVE ops + output DMA

        def emit_pass(s, ci2):
            rtb = rtbs[s]
            rtbr = rtb.rearrange("p (c h lh) t j -> p c h lh t j", c=2, h=2)

            def xvh(h, l):  # matmul rhs half [128, 2, 255]
                return rtbr[:, ci2, h, :, l % 2, (l // 2) : (l // 2) + 255]

            def xv4(l):  # DVE operand [128, 2, 2, 255]
                return rtbr[:, ci2, :, :, l % 2, (l // 2) : (l // 2) + 255]

            # ---- layer 1 (PE: fully independent of prev pass tail) ----
            z12 = pz.tile([128, 2, 512], F32, tag="z")
            for l in range(4):
                for h in range(2):
                    nc.tensor.matmul(
                        z12[:, h, 0:510], l1w_t[:, l, :], xvh(h, l),
                        start=(l == 0), stop=(l == 3),
                    )
            h1 = mlp.tile([128, 2, 512], BF16, tag="h1", bufs=2)
            nc.scalar.activation(h1, z12, RELU, bias=b1_t[:, :])

            # ---- squared inputs for |x|^2 (flat contiguous bf16) ----
            i2 = mlp.tile([128, 4, 2, 256], BF16, tag="i2", bufs=2)
            rtbf = rtb.rearrange("p a t j -> p (a t j)")
            nc.vector.tensor_tensor(
                i2.rearrange("p a t j -> p (a t j)"),
                rtbf[:, 2048 * ci2 : 2048 * (ci2 + 1)],
                rtbf[:, 2048 * ci2 : 2048 * (ci2 + 1)],
                MULT,
            )

            # ---- previous pass's tail: cty mms (PE), rsqrt+sim (Act/DVE) ----
            for fn in pending_tail:
                fn()
            pending_tail.clear()
            # ---- finished sample's fold front half (DMA/Pool/Act) ----
            for fn in pending_fold:
                fn()
            pending_fold.clear()

            # ---- layer 2 (reuses z12 banks after h1 consumed them) ----
            for h in range(2):
                nc.tensor.matmul(
                    z12[:, h, 0:510], l2w_t[:, :], h1[:, h, 0:510],
                    start=True, stop=True,
                )
            h2 = mlp.tile([128, 2, 512], BF16, tag="h2", bufs=2)
            nc.scalar.activation(h2, z12, RELU, bias=b2_t[:, :])

            # ---- contraction PSUM tile [128=(32*q+g), 3=(xy,|x|^2,|y|^2), 256]
            # quarter q=(2h+lh) puts each 255-col quarter of the pass on its
            # own partition range so the cosine stage runs 128 lanes wide
            ct = pct.tile([128, 3, 256], F32, tag="ct")

            def z3mm(lw):
                z3 = pz.tile([128, 2, 512], F32, tag="z")
                for h in range(2):
                    nc.tensor.matmul(
                        z3[:, h, 0:510], l3w_t[:, lw, :], h2[:, h, 0:510],
                        start=True, stop=True,
                    )
                return z3

            z3 = z3mm(0)
            # |x|^2 block: only needs i2 — dense PE filler while yv-l0 runs
            i2r = i2.rearrange("p (h lh) t j -> p h lh t j", h=2)
            for ll in range(4):
                for h in range(2):
                    for lh in range(2):
                        nc.tensor.matmul(
                            ct[32 * (2 * h + lh) : 32 * (2 * h + lh) + 32, 1, 0:255],
                            cw_t,
                            i2r[:, h, lh, ll % 2, (ll // 2) : (ll // 2) + 255],
                            start=(ll == 0), stop=(ll == 3),
                            tile_position=(0, 32 * (2 * h + lh)),
                        )

            ysqs = []
            for l in range(4):
                # full-width contiguous act read; yv keeps z3's packed layout
                # (valid halves are 510 = (lh 2, j 255) packed + 2 pad cols)
                yv = mlp.tile([128, 2, 512], BF16, tag="yv", bufs=4)
                nc.scalar.activation(yv, z3, RELU, bias=b3_t[:, l : l + 1])
                prod = mlp.tile([128, 2, 2, 255], BF16, tag="prod", bufs=2)
                nc.vector.tensor_tensor(
                    prod,
                    yv[:, :, 0:510].rearrange("p h (lh j) -> p h lh j", lh=2),
                    xv4(l), MULT,
                )
                ysq = mlp.tile([128, 2, 512], BF16, tag=f"ysq{l}", bufs=2)
                eng = nc.gpsimd if l < 3 else nc.vector
                eng.tensor_tensor(
                    ysq.rearrange("p h x -> p (h x)"),
                    yv.rearrange("p h x -> p (h x)"),
                    yv.rearrange("p h x -> p (h x)"),
                    MULT,
                )
                ysqs.append(ysq)
                if l < 3:
                    z3 = z3mm(l + 1)  # next l's z3 (deps via pool rotation)
                for h in range(2):
                    for lh in range(2):
                        nc.tensor.matmul(
                            ct[32 * (2 * h + lh) : 32 * (2 * h + lh) + 32, 0, 0:255],
                            cw_t, prod[:, h, lh, :],
                            start=(l == 0), stop=(l == 3),
                            tile_position=(0, 32 * (2 * h + lh)),
                        )

            sim32 = sim32s[s]

            def tail(ct=ct, ysqs=ysqs, sim32=sim32, ci2=ci2):
                for l in range(4):
                    for h in range(2):
                        for lh in range(2):
                            nc.tensor.matmul(
                                ct[32 * (2 * h + lh) : 32 * (2 * h + lh) + 32, 2, 0:255],
                                cw_t,
                                ysqs[l][:, h, 255 * lh : 255 * lh + 255],
                                start=(l == 0), stop=(l == 3),
                                tile_position=(0, 32 * (2 * h + lh)),
                            )
                # 1/(4*sqrt(ctx*cty)) = Rsqrt(16*ctx) * Rsqrt(cty)
                rs1 = simp.tile([128, 256], F32, tag="rs1")
                raw_activation(nc, rs1, ct[:, 1, :], RSQRT, bias=eps_t[:, :], scale=16.0)
                rs2 = simp.tile([128, 256], F32, tag="rs2")
                raw_activation(nc, rs2, ct[:, 2, :], RSQRT, bias=eps_t[:, :])
                st = simp.tile([128, 256], F32, tag="st")
                nc.vector.tensor_tensor(st, ct[:, 0, :], rs1, MULT)
                nc.vector.tensor_tensor(sim32[:, ci2, :], st, rs2, MULT)

            pending_tail.append(tail)

        def emit_fold(s):
            """Bounce S through DRAM into row-pair layout and fold. Emitted
            one pass after sample s's last sim write; the upsample half is
            deferred one further step via pending_up."""
            sim32 = sim32s[s]
            # S row-major in DRAM, row pitch 256: row(8g+4ci2+pq) from
            # sim32 partition 32pq+g, free (ci2, j). One DMA per quarter pq.
            sdram = dram.tile([256 * 256], F32, tag="sd")
            for pq in range(4):
                nc.sync.dma_start(
                    out=bass.AP(
                        tensor=sdram.tensor,
                        offset=sdram.offset + 256 * pq,
                        ap=[[8 * 256, 32], [4 * 256, 2], [1, 256]],
                    ),
                    in_=sim32[32 * pq : 32 * pq + 32],
                )
            # partition p holds S rows 2p,2p+1 (cols 0..254)
            simt = foldp.tile([128, 2, 256], F32, tag="simt")
            nc.sync.dma_start(
                out=simt[0:128, :, 0:255],
                in_=bass.AP(
                    tensor=sdram.tensor,
                    offset=sdram.offset,
                    ap=[[512, 128], [256, 2], [1, 255]],
                ),
            )
            # col fold R[i,v] = S[i,v-1]+S[i,v], edges doubled
            rf = foldp.tile([128, 2, 256], F32, tag="rf")
            nc.gpsimd.tensor_tensor(
                rf[:, :, 1:255], simt[:, :, 0:254], simt[:, :, 1:255], ADD
            )
            nc.scalar.activation(rf[:, :, 0:1], simt[:, :, 0:1], COPY, scale=2.0)
            nc.scalar.activation(rf[:, :, 255:256], simt[:, :, 254:255], COPY, scale=2.0)
            # S row 255 doesn't exist -> duplicate row 254 so T[255]=2*R[254]
            nc.sync.dma_start(out=rf[127:128, 1, :], in_=rf[127:128, 0, :])
            # partition-shifted copy of odd rows: rfs[q] = R[2q-1] (rfs[0]=R[0])
            rfs = foldp.tile([128, 256], F32, tag="rfs")
            nc.sync.dma_start(out=rfs[1:128, :], in_=rf[0:127, 1, :])
            nc.sync.dma_start(out=rfs[0:1, :], in_=rf[0:1, 0, :])

            def up_ops(rf=rf, rfs=rfs, s=s):
                # row fold + 2x2 upsample fused via stride-0 broadcast adds
                # up[p, lu, ru, v, cv] -> out row 4p+2lu+ru, col 2v+cv
                up = foldp.tile([128, 2, 2, 256, 2], F32, tag="up")

                def bc(a):
                    return a.unsqueeze(1).unsqueeze(3).to_broadcast([128, 2, 256, 2])

                nc.vector.tensor_tensor(up[:, 1], bc(rf[:, 0, :]), bc(rf[:, 1, :]), ADD)
                nc.vector.tensor_tensor(up[:, 0], bc(rfs[:, :]), bc(rf[:, 0, :]), ADD)
                nc.sync.dma_start(
                    out=bass.AP(
                        tensor=out4.tensor,
                        offset=out4.offset + s * IMG * IMG,
                        ap=[[4 * IMG, 128], [2 * IMG, 2], [IMG, 2], [2, 256], [1, 2]],
                    ),
                    in_=up,
                )

            pending_up.append(up_ops)

        # ---------------- main schedule ----------------
        for s in range(NSAMP):
            sim32s[s] = simp.tile([128, 2, 256], F32, tag="sim32", name="sim32")
            if s >= 1 and s + 1 < NSAMP:
                load_sample(s + 1)
            for ci2 in range(2):
                emit_pass(s, ci2)
                if pending_up:
                    for fn in pending_up:
                        fn()
                    pending_up.clear()
                if s > 0 and ci2 == 0:
                    emit_fold(s - 1)
        for fn in pending_tail:
            fn()
        pending_tail.clear()
        emit_fold(NSAMP - 1)
        for fn in pending_up:
            fn()
        pending_up.clear()

    nc.finalize()
    return nc


def make_weight_inputs(W1, b1, W2, b2, W3, b3):
    """Host-side block-diagonal weight construction (all fp32)."""
    W1 = np.asarray(W1, np.float32)
    W2 = np.asarray(W2, np.float32)
    W3 = np.asarray(W3, np.float32)
    b1 = np.asarray(b1, np.float32)
    b2 = np.asarray(b2, np.float32)
    b3 = np.asarray(b3, np.float32)
    # partition orders: image/z3 rows p = 32k+g ; h1/h2 rows q = 32c+g
    l1w = np.zeros((128, 4, 128), np.float32)
    l2w = np.zeros((128, 128), np.float32)
    l3w = np.zeros((128, 4, 128), np.float32)
    b3v = np.zeros((128, 4), np.float32)
    cwm = np.zeros((128, 32), np.float32)
    for g in range(32):
        for l in range(4):
            for k in range(4):
                for c in range(4):
                    l1w[32 * k + g, l, 32 * c + g] = W1[4 * k + l, c]
                    l3w[32 * c + g, l, 32 * k + g] = W3[c, 4 * k + l]
                b3v[32 * k + g, l] = b3[4 * k + l]
                cwm[32 * k + g, g] = 1.0
        for c in range(4):
            for c2 in range(4):
                l2w[32 * c + g, 32 * c2 + g] = W2[c, c2]
    b1v = np.repeat(b1, 32).reshape(128, 1).astype(np.float32)
    b2v = np.repeat(b2, 32).reshape(128, 1).astype(np.float32)
    import ml_dtypes

    bf = ml_dtypes.bfloat16
    return {
        "l1w": l1w.astype(bf), "l2w": l2w.astype(bf), "l3w": l3w.astype(bf),
        "b3v": b3v, "cw": cwm.astype(bf), "b1v": b1v, "b2v": b2v,
    }


_NC = None


def get_nc():
    global _NC
    if _NC is None:
        _NC = build_nc()
    return _NC


def _bf16():
    import ml_dtypes

    return ml_dtypes.bfloat16


def gather_rows(img_n):
    """(n,512,512) f32 -> (n,128,4096) bf16: row-gathered and col-parity
    de-interleaved on-chip layout [p, li, t, jj] with col = 2jj+t."""
    n = img_n.shape[0]
    pad = np.zeros((n, IMG + 4, IMG), np.float32)
    pad[:, :IMG, :] = img_n
    p = np.arange(128)
    li = np.arange(8)
    rows_idx = 16 * (p[:, None] % 32) + (p[:, None] // 32) + 2 * li[None, :]
    out = pad[:, rows_idx, :]  # (n,128,8,512)
    out = out.reshape(n, 128, 8, 256, 2).transpose(0, 1, 2, 4, 3)  # (n,128,8,2,256)
    return np.ascontiguousarray(out.reshape(n, 128, 8 * IMG)).astype(_bf16())


def kernel(img, W1, b1, W2, b2, W3, b3):
    from concourse.bass_utils import run_bass_kernel_spmd

    img = np.asarray(img, np.float32).reshape(32, IMG, IMG)
    wts = make_weight_inputs(W1, b1, W2, b2, W3, b3)
    nc = get_nc()
    core_ids = list(range(NCORES))
    in_maps = []
    for c in range(NCORES):
        m = {"img4b": gather_rows(img[c * NSAMP : (c + 1) * NSAMP])}
        m.update(wts)
        in_maps.append(m)
    res = run_bass_kernel_spmd(nc, in_maps, core_ids)
    out = np.concatenate([res.results[i]["out4"] for i in range(NCORES)], axis=0)
    return out.astype(np.float32)


# revision 29
# speedup vs baseline: 1.2968x; 1.0467x over previous
"""Trainium2 Bass kernel for nn_Classical_autoencoder (patch MLP autoencoder + cosine fold).

Contract: kernel(**inputs) takes FULL inputs (img (32,1,512,512), W1 (16,4), b1 (4,),
W2 (4,4), b2 (4,), W3 (4,16), b3 (16,)) and returns the FULL (32,512,512) output.
Internally: pure data-parallel over 8 NeuronCores, 4 images per core.

Math (per image):
  patches x = im2col(img, 4x4, stride 2)           # (255*255, 16)
  y = relu(relu(relu(x@W1+b1)@W2+b2)@W3+b3)        # (P, 16)
  S[i,j] = x.y / (max(|x|,eps)*max(|y|,eps))       # (255,255)
  out[r,c] = mean of S[i,j] for i in {r//2-1, r//2} & [0,255), j likewise
  (the overlapping fold with k=4,s=2 reduces exactly to this 2-tap box filter
   on S, upsampled 2x with 2x2-constant blocks)

Layout on chip (per image):
  row tile RT [128=(32k+g), 8=(li), 2=(t), 256=(jj)] : partition (k,g) holds
      img row 16g+k+2li, de-interleaved by column parity (col = 2jj+t);
  patch (i=8g+li, j) kernel col l lives at t=l%2, jj=j+l//2.
  MLP runs with patches as matmul free dim, 32 groups block-diag weights;
  per-patch contractions (x.y, |x|^2, |y|^2) are ones-block-diag matmuls into
  one PSUM tile [96=(3 x 32g), ...]; cosine via direct-emitted Rsqrt act.
  Two 1020-column passes per image (li quartets); each matmul is split into
  two 510-wide halves (PSUM bank limit); each pass's similarity tail is
  software-pipelined into the next pass so the PE never drains.
"""

import sys

for _p in ("/opt/trn_rl_repo", "/root/.axon_site/_ro/trn_rl_repo"):
    if _p not in sys.path:
        sys.path.append(_p)

from contextlib import ExitStack

import numpy as np

import concourse.bass as bass
import concourse.tile as tile
from concourse import bacc, mybir

F32 = mybir.dt.float32
BF16 = mybir.dt.bfloat16

IMG = 512
KS = 4
STRIDE = 2
OH = 255  # output patches per dim
NSAMP = 4  # images per core
NCORES = 8


def raw_activation(nc, out, in_, func, bias=0.0, scale=1.0):
    """nc.scalar.activation minus the Rsqrt accuracy guard (measured 4e-5 max
    rel err on HW, far inside this problem's 2e-2 gate)."""
    eng = nc.scalar
    if isinstance(bias, float):
        bias = nc.const_aps.scalar_like(bias, in_)
    inputs = [eng.lower_ap(in_)]
    for arg in (bias, scale, 0.0):
        if isinstance(arg, bass.AP):
            inputs.append(eng.lower_ap(arg))
        else:
            inputs.append(mybir.ImmediateValue(dtype=mybir.dt.float32, value=arg))
    return eng.add_instruction(
        mybir.InstActivation(
            name=nc.get_next_instruction_name(),
            func=func,
            ins=inputs,
            outs=[eng.lower_ap(out)],
        )
    )


def build_nc() -> bass.Bass:
    nc = bacc.Bacc()

    img4b = nc.declare_dram_parameter("img4b", [NSAMP, 128, 2 * 8 * IMG], BF16, isOutput=False)[:]
    l1w = nc.declare_dram_parameter("l1w", [128, 4, 128], BF16, isOutput=False)[:]
    l2w = nc.declare_dram_parameter("l2w", [128, 128], BF16, isOutput=False)[:]
    l3w = nc.declare_dram_parameter("l3w", [128, 4, 128], BF16, isOutput=False)[:]
    b3v = nc.declare_dram_parameter("b3v", [128, 4], F32, isOutput=False)[:]
    cw = nc.declare_dram_parameter("cw", [128, 32], BF16, isOutput=False)[:]
    b1v = nc.declare_dram_parameter("b1v", [128, 1], F32, isOutput=False)[:]
    b2v = nc.declare_dram_parameter("b2v", [128, 1], F32, isOutput=False)[:]
    out4 = nc.declare_dram_parameter("out4", [NSAMP, IMG, IMG], F32, isOutput=True)[:]

    RELU = mybir.ActivationFunctionType.Relu
    COPY = mybir.ActivationFunctionType.Copy
    RSQRT = mybir.ActivationFunctionType.Rsqrt
    MULT = mybir.AluOpType.mult
    ADD = mybir.AluOpType.add

    with ExitStack() as ctx:
        tc = ctx.enter_context(tile.TileContext(nc))
        consts = ctx.enter_context(tc.tile_pool(name="consts", bufs=1))
        rows = ctx.enter_context(tc.tile_pool(name="rows", bufs=2))
        mlp = ctx.enter_context(tc.tile_pool(name="mlp", bufs=4))
        simp = ctx.enter_context(tc.tile_pool(name="simp", bufs=2))
        foldp = ctx.enter_context(tc.tile_pool(name="foldp", bufs=2))
        pz = ctx.enter_context(tc.tile_pool(name="pz", bufs=2, space="PSUM"))
        pct = ctx.enter_context(tc.tile_pool(name="pct", bufs=2, space="PSUM"))
        dram = ctx.enter_context(tc.tile_pool(name="dram", bufs=2, space="DRAM"))

        # ---- constants ----
        l1w_t = consts.tile([128, 4, 128], BF16)
        nc.sync.dma_start(out=l1w_t, in_=l1w[:, :, :])
        l2w_t = consts.tile([128, 128], BF16)
        nc.sync.dma_start(out=l2w_t, in_=l2w[:, :])
        l3w_t = consts.tile([128, 4, 128], BF16)
        nc.sync.dma_start(out=l3w_t, in_=l3w[:, :, :])
        b3_t = consts.tile([128, 4], F32)
        nc.sync.dma_start(out=b3_t, in_=b3v[:, :])
        cw_t = consts.tile([128, 32], BF16)
        nc.sync.dma_start(out=cw_t, in_=cw[:, :])
        b1_t = consts.tile([128, 1], F32)
        nc.sync.dma_start(out=b1_t, in_=b1v[:, :])
        b2_t = consts.tile([128, 1], F32)
        nc.sync.dma_start(out=b2_t, in_=b2v[:, :])
        eps_t = consts.tile([128, 1], F32)
        nc.vector.memset(eps_t, 1e-20)

        rtbs = [None] * NSAMP
        sim32s = [None] * NSAMP

        def load_sample(s):
            # slot 0: jj base 0; slot 1: shifted by one column pair so the
            # l=2,3 patch views start 4B-aligned (DVE 2x path needs it)
            rtb = rows.tile([128, 2, 8, 2, 256], BF16, tag="rtb", bufs=2)
            nc.sync.dma_start(
                out=rtb.rearrange("p w a t j -> p (w a t j)"), in_=img4b[s, :, :]
            )
            rtbs[s] = rtb

        load_sample(0)
        load_sample(1)

        # ---------------- deferred emission state ----------------
        pending_tail = []  # closures: prev pass's cty mms + rsqrt/sim tail
        pending_fold = []  # closures: finished sample's bounce + fold setup
        pending_up = []    # closures: fold's upsample DVE ops + output DMA

        def emit_pass(s, ci2):
            rtb = rtbs[s]
            rtbr = rtb.rearrange("p w (c h lh) t j -> p w c h lh t j", c=2, h=2)

            def xvh(h, l):  # matmul rhs half [128, 2, 255], always aligned
                return rtbr[:, l // 2, ci2, h, :, l % 2, 0:255]

            def xv4(l):  # DVE operand [128, 2, 2, 255], always aligned
                return rtbr[:, l // 2, ci2, :, :, l % 2, 0:255]

            # ---- layer 1 (PE: fully independent of prev pass tail) ----
            z12 = pz.tile([128, 2, 512], F32, tag="z")
            for l in range(4):
                for h in range(2):
                    nc.tensor.matmul(
                        z12[:, h, 0:510], l1w_t[:, l, :], xvh(h, l),
                        start=(l == 0), stop=(l == 3),
                    )
            h1 = mlp.tile([128, 2, 512], BF16, tag="h1", bufs=2)
            nc.scalar.activation(h1, z12, RELU, bias=b1_t[:, :])

            # ---- squared inputs for |x|^2 (flat contiguous bf16) ----
            i2 = mlp.tile([128, 4, 2, 256], BF16, tag="i2", bufs=2)
            rtbf = rtb.rearrange("p w a t j -> p (w a t j)")
            nc.vector.tensor_tensor(
                i2.rearrange("p a t j -> p (a t j)"),
                rtbf[:, 2048 * ci2 : 2048 * (ci2 + 1)],
                rtbf[:, 2048 * ci2 : 2048 * (ci2 + 1)],
                MULT,
            )

            # ---- previous pass's tail: cty mms (PE), rsqrt+sim (Act/DVE) ----
            for fn in pending_tail:
                fn()
            pending_tail.clear()
            # ---- finished sample's fold front half (DMA/Pool/Act) ----
            for fn in pending_fold:
                fn()
            pending_fold.clear()

            # ---- layer 2 (reuses z12 banks after h1 consumed them) ----
            for h in range(2):
                nc.tensor.matmul(
                    z12[:, h, 0:510], l2w_t[:, :], h1[:, h, 0:510],
                    start=True, stop=True,
                )
            h2 = mlp.tile([128, 2, 512], BF16, tag="h2", bufs=2)
            nc.scalar.activation(h2, z12, RELU, bias=b2_t[:, :])

            # ---- contraction PSUM tile [128=(32*q+g), 3=(xy,|x|^2,|y|^2), 256]
            # quarter q=(2h+lh) puts each 255-col quarter of the pass on its
            # own partition range so the cosine stage runs 128 lanes wide
            ct = pct.tile([128, 3, 256], F32, tag="ct")

            def z3mm(lw):
                z3 = pz.tile([128, 2, 512], F32, tag="z")
                for h in range(2):
                    nc.tensor.matmul(
                        z3[:, h, 0:510], l3w_t[:, lw, :], h2[:, h, 0:510],
                        start=True, stop=True,
                    )
                return z3

            z3 = z3mm(0)
            # ---- pure z3<->yv ladder; all contraction mms deferred to tail
            ysqs = []
            prods = []
            for l in range(4):
                # full-width contiguous act read; yv keeps z3's packed layout
                # (valid halves are 510 = (lh 2, j 255) packed + 2 pad cols)
                yv = mlp.tile([128, 2, 512], BF16, tag="yv", bufs=4)
                nc.scalar.activation(yv, z3, RELU, bias=b3_t[:, l : l + 1])
                if l < 3:
                    z3 = z3mm(l + 1)  # next l's z3 (deps via pool rotation)
                prod = mlp.tile([128, 2, 2, 255], BF16, tag=f"prod{l}", bufs=2)
                nc.vector.tensor_tensor(
                    prod,
                    yv[:, :, 0:510].rearrange("p h (lh j) -> p h lh j", lh=2),
                    xv4(l), MULT,
                )
                prods.append(prod)
                ysq = mlp.tile([128, 2, 512], BF16, tag=f"ysq{l}", bufs=2)
                eng = nc.gpsimd if l < 2 else nc.vector
                eng.tensor_tensor(
                    ysq.rearrange("p h x -> p (h x)"),
                    yv.rearrange("p h x -> p (h x)"),
                    yv.rearrange("p h x -> p (h x)"),
                    MULT,
                )
                ysqs.append(ysq)

            sim32 = sim32s[s]
            i2r = i2.rearrange("p (h lh) t j -> p h lh t j", h=2)

            def tail(ct=ct, ysqs=ysqs, prods=prods, i2r=i2r, sim32=sim32, ci2=ci2):
                # dense contraction block: all operands ready; runs at the top
                # of the next pass interleaved after its L1 so the PE stays hot
                for l in range(4):
                    for h in range(2):
                        for lh in range(2):
                            q = 2 * h + lh
                            nc.tensor.matmul(
                                ct[32 * q : 32 * q + 32, 0, 0:255],
                                cw_t, prods[l][:, h, lh, :],
                                start=(l == 0), stop=(l == 3),
                                tile_position=(0, 32 * q),
                            )
                for ll in range(4):
                    for h in range(2):
                        for lh in range(2):
                            q = 2 * h + lh
                            nc.tensor.matmul(
                                ct[32 * q : 32 * q + 32, 1, 0:255],
                                cw_t,
                                i2r[:, h, lh, ll % 2, (ll // 2) : (ll // 2) + 255],
                                start=(ll == 0), stop=(ll == 3),
                                tile_position=(0, 32 * q),
                            )
                for l in range(4):
                    for h in range(2):
                        for lh in range(2):
                            q = 2 * h + lh
                            nc.tensor.matmul(
                                ct[32 * q : 32 * q + 32, 2, 0:255],
                                cw_t,
                                ysqs[l][:, h, 255 * lh : 255 * lh + 255],
                                start=(l == 0), stop=(l == 3),
                                tile_position=(0, 32 * q),
                            )
                # 1/(4*sqrt(ctx*cty)) = Rsqrt(16*ctx) * Rsqrt(cty)
                rs1 = simp.tile([128, 256], F32, tag="rs1")
                raw_activation(nc, rs1, ct[:, 1, :], RSQRT, bias=eps_t[:, :], scale=16.0)
                rs2 = simp.tile([128, 256], F32, tag="rs2")
                raw_activation(nc, rs2, ct[:, 2, :], RSQRT, bias=eps_t[:, :])
                st = simp.tile([128, 256], F32, tag="st")
                nc.vector.tensor_tensor(st, ct[:, 0, :], rs1, MULT)
                nc.vector.tensor_tensor(sim32[:, ci2, :], st, rs2, MULT)

            pending_tail.append(tail)

        def emit_fold(s):
            """Bounce S through DRAM into row-pair layout and fold. Emitted
            one pass after sample s's last sim write; the upsample half is
            deferred one further step via pending_up."""
            sim32 = sim32s[s]
            # S row-major in DRAM, row pitch 256: row(8g+4ci2+pq) from
            # sim32 partition 32pq+g, free (ci2, j). One DMA per quarter pq.
            sdram = dram.tile([256 * 256], F32, tag="sd")
            for pq in range(4):
                nc.sync.dma_start(
                    out=bass.AP(
                        tensor=sdram.tensor,
                        offset=sdram.offset + 256 * pq,
                        ap=[[8 * 256, 32], [4 * 256, 2], [1, 256]],
                    ),
                    in_=sim32[32 * pq : 32 * pq + 32],
                )
            # partition p holds S rows 2p,2p+1 (cols 0..254)
            simt = foldp.tile([128, 2, 256], F32, tag="simt")
            nc.sync.dma_start(
                out=simt[0:128, :, 0:255],
                in_=bass.AP(
                    tensor=sdram.tensor,
                    offset=sdram.offset,
                    ap=[[512, 128], [256, 2], [1, 255]],
                ),
            )
            # col fold R[i,v] = S[i,v-1]+S[i,v], edges doubled
            rf = foldp.tile([128, 2, 256], F32, tag="rf")
            nc.gpsimd.tensor_tensor(
                rf[:, :, 1:255], simt[:, :, 0:254], simt[:, :, 1:255], ADD
            )
            nc.scalar.activation(rf[:, :, 0:1], simt[:, :, 0:1], COPY, scale=2.0)
            nc.scalar.activation(rf[:, :, 255:256], simt[:, :, 254:255], COPY, scale=2.0)
            # S row 255 doesn't exist -> duplicate row 254 so T[255]=2*R[254]
            nc.sync.dma_start(out=rf[127:128, 1, :], in_=rf[127:128, 0, :])
            # partition-shifted copy of odd rows: rfs[q] = R[2q-1] (rfs[0]=R[0])
            rfs = foldp.tile([128, 256], F32, tag="rfs")
            nc.sync.dma_start(out=rfs[1:128, :], in_=rf[0:127, 1, :])
            nc.sync.dma_start(out=rfs[0:1, :], in_=rf[0:1, 0, :])

            def up_ops(rf=rf, rfs=rfs, s=s):
                # row fold + 2x2 upsample fused via stride-0 broadcast adds
                # up[p, lu, ru, v, cv] -> out row 4p+2lu+ru, col 2v+cv
                up = foldp.tile([128, 2, 2, 256, 2], F32, tag="up")

                def bc(a):
                    return a.unsqueeze(1).unsqueeze(3).to_broadcast([128, 2, 256, 2])

                nc.vector.tensor_tensor(up[:, 1], bc(rf[:, 0, :]), bc(rf[:, 1, :]), ADD)
                nc.vector.tensor_tensor(up[:, 0], bc(rfs[:, :]), bc(rf[:, 0, :]), ADD)
                nc.sync.dma_start(
                    out=bass.AP(
                        tensor=out4.tensor,
                        offset=out4.offset + s * IMG * IMG,
                        ap=[[4 * IMG, 128], [2 * IMG, 2], [IMG, 2], [2, 256], [1, 2]],
                    ),
                    in_=up,
                )

            pending_up.append(up_ops)

        # ---------------- main schedule ----------------
        for s in range(NSAMP):
            sim32s[s] = simp.tile([128, 2, 256], F32, tag="sim32", name="sim32")
            if s >= 1 and s + 1 < NSAMP:
                load_sample(s + 1)
            for ci2 in range(2):
                emit_pass(s, ci2)
                if pending_up:
                    for fn in pending_up:
                        fn()
                    pending_up.clear()
                if s > 0 and ci2 == 0:
                    emit_fold(s - 1)
        for fn in pending_tail:
            fn()
        pending_tail.clear()
        emit_fold(NSAMP - 1)
        for fn in pending_up:
            fn()
        pending_up.clear()

    nc.finalize()
    return nc


def make_weight_inputs(W1, b1, W2, b2, W3, b3):
    """Host-side block-diagonal weight construction (all fp32)."""
    W1 = np.asarray(W1, np.float32)
    W2 = np.asarray(W2, np.float32)
    W3 = np.asarray(W3, np.float32)
    b1 = np.asarray(b1, np.float32)
    b2 = np.asarray(b2, np.float32)
    b3 = np.asarray(b3, np.float32)
    # partition orders: image/z3 rows p = 32k+g ; h1/h2 rows q = 32c+g
    l1w = np.zeros((128, 4, 128), np.float32)
    l2w = np.zeros((128, 128), np.float32)
    l3w = np.zeros((128, 4, 128), np.float32)
    b3v = np.zeros((128, 4), np.float32)
    cwm = np.zeros((128, 32), np.float32)
    for g in range(32):
        for l in range(4):
            for k in range(4):
                for c in range(4):
                    l1w[32 * k + g, l, 32 * c + g] = W1[4 * k + l, c]
                    l3w[32 * c + g, l, 32 * k + g] = W3[c, 4 * k + l]
                b3v[32 * k + g, l] = b3[4 * k + l]
                cwm[32 * k + g, g] = 1.0
        for c in range(4):
            for c2 in range(4):
                l2w[32 * c + g, 32 * c2 + g] = W2[c, c2]
    b1v = np.repeat(b1, 32).reshape(128, 1).astype(np.float32)
    b2v = np.repeat(b2, 32).reshape(128, 1).astype(np.float32)
    import ml_dtypes

    bf = ml_dtypes.bfloat16
    return {
        "l1w": l1w.astype(bf), "l2w": l2w.astype(bf), "l3w": l3w.astype(bf),
        "b3v": b3v, "cw": cwm.astype(bf), "b1v": b1v, "b2v": b2v,
    }


_NC = None


def get_nc():
    global _NC
    if _NC is None:
        _NC = build_nc()
    return _NC


def _bf16():
    import ml_dtypes

    return ml_dtypes.bfloat16


def gather_rows(img_n):
    """(n,512,512) f32 -> (n,128,8192) bf16: row-gathered, col-parity
    de-interleaved layout [p, w, li, t, jj] with col = 2(jj+w)+t (slot w=1 is
    shifted one column-pair so every kernel-column view starts 4B-aligned)."""
    n = img_n.shape[0]
    pad = np.zeros((n, IMG + 4, IMG), np.float32)
    pad[:, :IMG, :] = img_n
    p = np.arange(128)
    li = np.arange(8)
    rows_idx = 16 * (p[:, None] % 32) + (p[:, None] // 32) + 2 * li[None, :]
    out = pad[:, rows_idx, :]  # (n,128,8,512)
    out = out.reshape(n, 128, 8, 256, 2).transpose(0, 1, 2, 4, 3)  # (n,128,8,2,256)
    sh = np.concatenate([out[..., 1:], np.zeros_like(out[..., :1])], axis=-1)
    both = np.stack([out, sh], axis=2)  # (n,128,2,8,2,256)
    return np.ascontiguousarray(both.reshape(n, 128, 2 * 8 * IMG)).astype(_bf16())


def kernel(img, W1, b1, W2, b2, W3, b3):
    from concourse.bass_utils import run_bass_kernel_spmd

    img = np.asarray(img, np.float32).reshape(32, IMG, IMG)
    wts = make_weight_inputs(W1, b1, W2, b2, W3, b3)
    nc = get_nc()
    core_ids = list(range(NCORES))
    in_maps = []
    for c in range(NCORES):
        m = {"img4b": gather_rows(img[c * NSAMP : (c + 1) * NSAMP])}
        m.update(wts)
        in_maps.append(m)
    res = run_bass_kernel_spmd(nc, in_maps, core_ids)
    out = np.concatenate([res.results[i]["out4"] for i in range(NCORES)], axis=0)
    return out.astype(np.float32)
